# revision 24
# baseline (speedup 1.0000x reference)
"""Trainium2 Bass kernel: batch biquad IIR as a truncated-FIR banded matmul.

The reference IIR y[t] = sum_m b[m] x[t-m]/a0 - sum_n a[n]/a0 y[t-n] has a
fast-decaying impulse response for this filter (poles at |z| = sqrt(0.1716)),
so y is computed exactly (to below-fp32-noise truncation, |h| < 1e-9) as a
J~25-tap FIR of x. This removes the time recurrence entirely: no DVE scan
(3 cycles/element serial — 82us/core for this shape), no feedback state.

Layout: each waveform row (65536 samples) is reshaped host-side to a
[128, 512] segment matrix X (partition k holds samples 128*f + k). Then

    y(128f + i) = sum_k W1[k, i] X[k, f] + sum_k W2[k, i] X[k, f-1]

with W1[k,i] = h[i-k] (banded lower Toeplitz) and W2[k,i] = h[i+128-k]
(corner band), i.e. TWO accumulating PE matmuls per 512-column group with
stationary weights. Rows are concatenated along the free axis; the one
cross-row halo column per row start is corrected on the host (the leak is
linear and only touches the first J-1 samples of each row).

Engine budget per core (8 tiles x 4096 cols): PE ~30us of fp16 matmuls
(1 cyc/col; fp32 would be 4x and PE-bound), PSUM->SBUF cast copies split
Scalar/DVE (~18us each; gpsimd cannot read PSUM), DMA 2x8.4MB fp16 ~43us
over 16 DGE engines at ~25GB/s each -> DMA-bound. Measured 58-64us
(varies with chip power throttling) vs the scan baseline's 116-128us;
~9.5us of that is a fixed backend teardown (per-semaphore resets) plus
~2us startup, both invariant to program shape.

Sharding: pure data parallel, 64 rows per core on 8 cores. I/O is fp16
(x cast host-side, y cast back): halves HBM traffic; error ~2.7e-3 max
absolute = 6.8e-4 of output scale vs the 2e-2 scale-relative gate.
"""

import numpy as np

# Problem geometry (hardcoded per the grading contract).
N_CORES = 8
BATCH = 512
T = 65536
ROWS = BATCH // N_CORES          # 64 rows per core
SEG = 128                        # samples per segment column (= partitions)
F = T // SEG                     # 512 segment columns per row
COLS = ROWS * F                  # 32768 free-axis columns per core
CW = 4096                        # tile width (free cols); 8 groups of 512
NT = COLS // CW                  # 8 tiles
GW = 512                         # matmul group cols (one PSUM bank)
HW_ = 1024                       # copy granularity (two banks)

# I/O + matmul precision mode: "fp16" | "bf16" | "fp32r"
IO_MODE = "fp16"


def _fir_taps(b, a, tol=1e-9, max_taps=120):
    """Impulse response of the IIR filter, truncated to J taps (float64)."""
    b = np.asarray(b, np.float64)
    a = np.asarray(a, np.float64)
    L = 256
    h = np.zeros(L)
    for t in range(L):
        acc = b[t] if t < len(b) else 0.0
        for n in range(1, len(a)):
            if t - n >= 0:
                acc -= a[n] * h[t - n]
        h[t] = acc / a[0]
    mag = np.abs(h)
    keep = np.nonzero(mag > tol * mag.max())[0]
    J = int(keep.max()) + 1
    assert J <= max_taps, f"impulse response too long for FIR approach: {J}"
    assert J <= SEG, J
    return h[:J]


def _np_dt(mode):
    return {"fp16": np.float16, "bf16": None, "fp32r": np.float32}[mode]


def _build_program(mode):
    import concourse.bacc as bacc
    import concourse.mybir as mybir
    import concourse.tile as tile

    dt_io = {
        "fp16": mybir.dt.float16,
        "bf16": mybir.dt.bfloat16,
        "fp32r": mybir.dt.float32r,
    }[mode]
    f32 = mybir.dt.float32

    nc = bacc.Bacc("TRN2", target_bir_lowering=False, debug=False)
    x = nc.dram_tensor("x", [NT, SEG, CW + 1], dt_io, kind="ExternalInput")
    w1 = nc.dram_tensor("w1", [SEG, SEG], dt_io, kind="ExternalInput")
    w2 = nc.dram_tensor("w2", [SEG, SEG], dt_io, kind="ExternalInput")
    y = nc.dram_tensor("y", [NT, SEG, CW], dt_io, kind="ExternalOutput")

    with tile.TileContext(nc) as tc:
        with (
            tc.tile_pool(name="const", bufs=1) as cpool,
            tc.tile_pool(name="xin", bufs=5) as xpool,
            tc.tile_pool(name="yout", bufs=4) as ypool,
            tc.tile_pool(name="ps", bufs=2, space="PSUM") as psum,
        ):
            # Weights go on the gpsimd queue: they land in parallel with the
            # first input segment (sync queue starts x immediately) and the
            # transfer itself warms the gpsimd DGE ring for the output
            # stream (a fresh ring adds ~4us issue->packet latency).
            wt1 = cpool.tile([SEG, SEG], dt_io, tag="w1")
            wt2 = cpool.tile([SEG, SEG], dt_io, tag="w2")
            nc.gpsimd.dma_start(out=wt1[:], in_=w1[:, :])
            nc.gpsimd.dma_start(out=wt2[:], in_=w2[:, :])

            # PE DVFS pre-ramp: the PE reaches full clock only after ~3us of
            # continuous execution (measured: 392ns vs 213ns per 512-col
            # fp16 matmul). Burn the idle window before the first input
            # lands with dummy matmuls on scratch data so the real matmul
            # stream runs at full speed from the start.
            scr = cpool.tile([SEG, GW], dt_io, tag="scr")
            sink = cpool.tile([SEG, 16], dt_io, tag="sink")
            nc.vector.memset(scr[:], 0.0)
            ptw = psum.tile([SEG, 2048], f32, tag="pt")
            for _ in range(8):
                nc.tensor.matmul(ptw[:, 0:GW], scr[:, 0:SEG], scr[:],
                                 start=True, stop=True)
            nc.vector.tensor_scalar_mul(sink[:], ptw[:, 0:16], 1.0)

            ncopy = 0
            for m in range(NT):
                xt = xpool.tile([SEG, CW + 1], dt_io, tag="xt")
                # Tile 0 loads in fine-grained segments so PE starts early.
                segw = [512, 512, 1024, 2048] if m == 0 else [CW]
                a_ = 0
                for w_ in segw:
                    b_ = a_ + w_ + (1 if a_ == 0 else 0)
                    nc.sync.dma_start(out=xt[:, a_:b_], in_=x[m, :, a_:b_])
                    a_ = b_

                yt = ypool.tile([SEG, CW], dt_io, tag="yt")
                # Per chunk: W1 pass, W2 pass (LDWEIGHTS double-buffers so
                # the swap hides), then a cast copy PSUM->SBUF and the
                # output DMA (gpsimd queue). Tile 0 uses fine chunks for an
                # early output start and the last tile for a short drain
                # tail; steady tiles use 2048-col chunks to cut instruction
                # and semaphore count (the copies must keep pace with the
                # 2.7us/tile input stream or the drain tail grows).
                if m == 0:
                    chunks = [512, 512, 1024, 1024, 1024]
                elif m == NT - 1:
                    chunks = [512] * 8
                else:
                    chunks = [2048, 2048]
                c0 = 0
                for cwid in chunks:
                    ptf = psum.tile([SEG, 2048], f32, tag="pt")
                    pt = ptf[:, :cwid]
                    for sub in range(cwid // GW or 1):
                        g0 = c0 + sub * GW
                        gw = min(GW, cwid)
                        nc.tensor.matmul(
                            pt[:, sub * gw : sub * gw + gw], wt1[:],
                            xt[:, 1 + g0 : 1 + g0 + gw],
                            start=True, stop=False)
                    for sub in range(cwid // GW or 1):
                        g0 = c0 + sub * GW
                        gw = min(GW, cwid)
                        nc.tensor.matmul(
                            pt[:, sub * gw : sub * gw + gw], wt2[:],
                            xt[:, g0 : g0 + gw],
                            start=False, stop=True)
                    # cast copy PSUM -> SBUF (gpsimd cannot read PSUM, so
                    # rotate scalar/vector only), then the output DMA on the
                    # copying engine's own queue (scalar) or gpsimd's: two
                    # queues drain the end-of-run output backlog in parallel.
                    dst = yt[:, c0 : c0 + cwid]
                    on_scalar = ncopy % 2 == 0
                    if on_scalar:
                        nc.scalar.copy(dst, pt[:])
                    else:
                        nc.vector.tensor_scalar_mul(dst, pt[:], 1.0)
                    ncopy += 1
                    c0 += cwid
                    qeng = nc.scalar if on_scalar else nc.gpsimd
                    qeng.dma_start(
                        out=y[m, :, c0 - cwid : c0], in_=yt[:, c0 - cwid : c0])

    nc.compile()
    return nc


_CACHE: dict = {}


def _get_program(mode):
    if mode not in _CACHE:
        _CACHE[mode] = _build_program(mode)
    return _CACHE[mode]


def _quant(arr, mode):
    """Cast host array to the device I/O dtype (numpy view of it)."""
    if mode == "fp16":
        return arr.astype(np.float16)
    if mode == "fp32r":
        return arr.astype(np.float32)
    if mode == "bf16":
        import ml_dtypes
        return arr.astype(ml_dtypes.bfloat16)
    raise ValueError(mode)


def _dequant(arr):
    return np.asarray(arr, dtype=np.float32)


def _weights(h, mode):
    J = len(h)
    W1 = np.zeros((SEG, SEG), np.float64)
    W2 = np.zeros((SEG, SEG), np.float64)
    for k in range(SEG):
        for i in range(SEG):
            d = i - k
            if 0 <= d < J:
                W1[k, i] = h[d]
            d2 = i + SEG - k
            if 1 <= d2 < J:
                W2[k, i] = h[d2]
    return _quant(W1, mode), _quant(W2, mode)


def run(x, b, a, trace: bool = False):
    """Run the kernel on the full (512, 65536) input; returns (y, exec_time_ns)."""
    from concourse.bass_utils import run_bass_kernel_spmd

    x = np.asarray(x, dtype=np.float32)
    assert x.shape == (BATCH, T), x.shape
    mode = IO_MODE
    h = _fir_taps(b, a)
    J = len(h)
    w1q, w2q = _weights(h, mode)
    nc = _get_program(mode)

    xq = _quant(x, mode)                       # (512, T) device-precision x
    in_maps = []
    for c in range(N_CORES):
        xc = xq[c * ROWS : (c + 1) * ROWS]     # (64, T)
        X = np.ascontiguousarray(
            xc.reshape(ROWS, F, SEG).transpose(2, 0, 1).reshape(SEG, COLS))
        Xp = np.concatenate([np.zeros((SEG, 1), X.dtype), X], axis=1)
        xd = np.stack([Xp[:, m * CW : m * CW + CW + 1] for m in range(NT)])
        in_maps.append({"x": xd, "w1": w1q, "w2": w2q})

    res = run_bass_kernel_spmd(nc, in_maps, list(range(N_CORES)), trace=trace)

    # Host-side fix of the one cross-row halo column per row: the device's
    # W2 matmul at each row's first group read the previous row's last
    # column. Subtract exactly what the device added (fp16 products are
    # exact in fp32, so recomputing with the quantized h/x matches PE).
    hq = _dequant(_quant(h, mode))             # (J,)
    NC_ = J - 1
    Hm = np.zeros((NC_, NC_), np.float32)      # Hm[d-1, i] = h[i+d]
    for dd in range(1, J):
        for i in range(0, J - dd):
            Hm[dd - 1, i] = hq[i + dd]

    out = np.empty((BATCH, T), dtype=np.float32)
    for c in range(N_CORES):
        yd = _dequant(res.results[c]["y"])     # (NT, SEG, CW)
        Y = yd.transpose(1, 0, 2).reshape(SEG, COLS)
        yc = np.ascontiguousarray(
            Y.reshape(SEG, ROWS, F).transpose(1, 2, 0).reshape(ROWS, T))
        xc = _dequant(xq[c * ROWS : (c + 1) * ROWS])
        # tail_rev[r, d-1] = x[r, T-d] for d in 1..J-1, rows 0..62
        tail_rev = xc[:-1, T - 1 : T - J : -1]           # (63, J-1)
        corr = tail_rev @ Hm                             # (63, J-1)
        yc[1:, : NC_] -= corr
        out[c * ROWS : (c + 1) * ROWS] = yc
    return out, res.exec_time_ns


def kernel(x, b, a):
    out, _ = run(x, b, a, trace=False)
    return out
